# revision 15
# baseline (speedup 1.0000x reference)
"""Trainium2 Bass kernel for nn_AttnResBlock (RMSNorm -> scalar proj ->
softmax over depth N -> weighted sum of history).

Reference computation (per (b, t) position, D=1024, N=13):
  ms      = mean_d(V^2)
  logits  = rsqrt(ms + eps) * sum_d(V * (rms_weight * w_proj))
  alpha   = softmax_n(logits)
  out     = sum_n alpha_n * V_n

Sharding: B*T = 4096 positions split contiguously across 8 cores (512
positions each). All reductions are over D and N, both core-local -> no
collectives. Tiny [D] params are replicated (pre-broadcast on host).

Per-core layout: partitions = positions (tiles of 128), free dim = (n, d).
V is pre-transposed on the host to [pos, n, d]. Each tile's V is held as
two half-tiles (n 0:7 / 7:13) in separate pools so SBUF slots recycle at
half-tile granularity (the PE frees the first half mid-tile).

Measured per-slice engine costs (HW): DVE fused STT+accum ~1.36us, ACT
Square+accum ~1.42us, GpSimd bulk elementwise ~2.5us (DSP ucode - only
small ops live there). The 26 fused reduce slices are split:
  DVE : 13 w_proj dots + 1 square, softmax logits/max/recip/alpha
  ACT : 12 squares, rsqrt via ln/exp, softmax exp, PSUM evacuation
  GPS : diag-bank build (except last tile: DVE, to shorten the tail),
        SWDGE load groups + output stores
  PE  : weighted sum over N as 13 accumulating diag-matmuls in float32r
"""

import numpy as np

import concourse.bacc as bacc
import concourse.hw_specs as hw_specs
import concourse.mybir as mybir
from concourse.bass import ts
from concourse.bass_utils import run_bass_kernel_spmd
from concourse.tile import TileContext

N, B, T, D = 13, 2, 2048, 1024
N_CORES = 8
POS_TOTAL = B * T                    # 4096
POS_PER_CORE = POS_TOTAL // N_CORES  # 512
P = 128                              # SBUF partitions = positions per tile
TILES = POS_PER_CORE // P            # 4
EPS = float(np.finfo(np.float32).eps)
NSPLIT = 7                           # slices 0:7 in half A, 7:13 in half B

F32 = mybir.dt.float32
F32R = mybir.dt.float32r
BF16 = mybir.dt.bfloat16
Alu = mybir.AluOpType
Act = mybir.ActivationFunctionType

# load groups (slice ranges of n, ring): ring 0=sync HWDGE, 1=gpsimd SWDGE
LOAD_GROUPS = [(0, 4, 0), (4, 7, 1), (7, 11, 0), (11, 13, 1)]
# squares on DVE (rest of squares on ACT; all 13 dots on DVE)
DVE_SQS = (5,)
MM_FREE = 512  # free dim per matmul (2 per slice); PSUM bank limit for 4B

_CACHE = {}

_ACT_SET = "natural_log_exp_and_others"  # contains ln, exp, square, copy


def _patched_tables(orig):
    def fn(module_arch):
        t = orig(module_arch)
        return {k: (v if k == _ACT_SET else set()) for k, v in t.items()}

    return fn


def _build():
    nc = bacc.Bacc(None, target_bir_lowering=False)
    v = nc.dram_tensor("v", [POS_PER_CORE, N, D], F32R, kind="ExternalInput")
    wcb = nc.dram_tensor("wcb", [P, D], F32, kind="ExternalInput")
    identb = nc.dram_tensor("identb", [P, N, P], BF16, kind="ExternalInput")
    o = nc.dram_tensor("o", [POS_PER_CORE, D], F32, kind="ExternalOutput")

    with TileContext(nc) as tc:
        with (
            tc.tile_pool(name="cst", bufs=1) as cst,
            tc.tile_pool(name="vpa", bufs=3) as vpa,
            tc.tile_pool(name="vpb", bufs=3) as vpb,
            tc.tile_pool(name="sm", bufs=3) as sm,
            tc.tile_pool(name="dg", bufs=2) as dg,
            tc.tile_pool(name="ob", bufs=2) as ob,
            tc.tile_pool(name="ps", bufs=3, space="PSUM") as psp,
        ):
            wct = cst.tile([P, D], F32)
            idb = cst.tile([P, N, P], BF16)
            epst = cst.tile([P, 1], F32)

            rings = [nc.sync, nc.gpsimd]
            # wct gates every DVE dot -> absolutely first on the sync ring;
            # idb only gates the PE stage -> scalar ring (otherwise idle)
            nc.sync.dma_start(out=wct[:], in_=wcb[:, :])
            nc.scalar.dma_start(out=idb[:], in_=identb[:, :, :])
            nc.vector.memset(epst[:], EPS)
            pending_store = None  # deferred (psum, tile_idx) evac+store

            def vslice(halves, n):
                va, vb = halves
                return va[:, n, :] if n < NSPLIT else vb[:, n - NSPLIT, :]

            for t in range(TILES):
                # ---- load [128 pos, 13 n, 1024 d] as two half-tiles ----
                va = vpa.tile([P, NSPLIT, D], F32R, tag="va")
                vb = vpb.tile([P, N - NSPLIT, D], F32R, tag="vb")
                halves = (va, vb)
                for lo, hi, r in LOAD_GROUPS:
                    if hi <= NSPLIT:
                        dst = va[:, lo:hi, :]
                    else:
                        dst = vb[:, lo - NSPLIT : hi - NSPLIT, :]
                    rings[r].dma_start(out=dst, in_=v[ts(t, P), lo:hi, :])

                # previous tile's PSUM evacuation + store, deferred here so
                # it never blocks this tile's stats behind the PE in the
                # in-order engine queues
                if pending_store is not None:
                    pps, pt = pending_store
                    osb = ob.tile([P, D], F32, tag="osb")
                    nc.scalar.copy(osb[:], pps[:])
                    nc.gpsimd.dma_start(out=o[ts(pt, P), :], in_=osb[:])
                    pending_store = None

                dotv = sm.tile([P, N], F32, tag="dotv")
                msv = sm.tile([P, N], F32, tag="msv")
                dumv = sm.tile([P, 1], F32, tag="dumv")
                trash = sm.tile([P, D], F32, tag="trash")
                for n in range(N):
                    vtn = vslice(halves, n).bitcast(F32)
                    # dot_n = sum_d V * (rms_weight * w_proj)   (fused DVE)
                    nc.vector.scalar_tensor_tensor(
                        out=dumv[:].broadcast_to((P, D)),
                        in0=vtn,
                        scalar=0.0,
                        in1=wct[:],
                        op0=Alu.bypass,
                        op1=Alu.mult,
                        accum_out=dotv[:, n : n + 1],
                    )
                    # ms_n = sum_d V^2
                    if n in DVE_SQS:
                        nc.vector.scalar_tensor_tensor(
                            out=dumv[:].broadcast_to((P, D)),
                            in0=vtn,
                            scalar=0.0,
                            in1=vtn,
                            op0=Alu.bypass,
                            op1=Alu.mult,
                            accum_out=msv[:, n : n + 1],
                        )
                    else:
                        nc.scalar.activation(
                            out=trash[:],
                            in_=vtn,
                            func=Act.Square,
                            accum_out=msv[:, n : n + 1],
                        )

                # ---- softmax over n (high priority: unblocks PE + buffers) ----
                with tc.high_priority(offset=100):
                    # rsqrt(mean + eps) = exp(-0.5 * ln(ms/D + eps))
                    lnv = sm.tile([P, N], F32, tag="lnv")
                    rsq = sm.tile([P, N], F32, tag="rsq")
                    nc.scalar.activation(
                        lnv[:], msv[:], Act.Ln, bias=epst[:], scale=1.0 / D
                    )
                    nc.scalar.activation(rsq[:], lnv[:], Act.Exp, scale=-0.5)
                    lg = sm.tile([P, N], F32, tag="lg")
                    nc.vector.tensor_tensor(lg[:], dotv[:], rsq[:], Alu.mult)
                    negm = sm.tile([P, 1], F32, tag="negm")
                    nc.vector.tensor_reduce(
                        out=negm[:], in_=lg[:], op=Alu.max,
                        axis=mybir.AxisListType.X, negate=True,
                    )
                    # exp with fused denominator accumulation
                    ev = sm.tile([P, N], F32, tag="ev")
                    ssum = sm.tile([P, 1], F32, tag="ssum")
                    nc.scalar.activation(
                        ev[:], lg[:], Act.Exp, bias=negm[:], accum_out=ssum[:]
                    )
                    rcp = sm.tile([P, 1], F32, tag="rcp")
                    nc.vector.reciprocal(rcp[:], ssum[:])
                    alpha = sm.tile([P, N, 1], F32, tag="alpha")
                    nc.vector.tensor_scalar(
                        alpha[:, :, 0], ev[:], rcp[:], None, Alu.mult
                    )

                    # ---- weighted sum over n: PE with diagonal-alpha stationary ----
                    dgb = dg.tile([P, N, P], F32R, tag="dgb")
                    dgb_eng = nc.vector if t == TILES - 1 else nc.gpsimd
                    dgb_eng.tensor_tensor(
                        dgb[:], idb[:], alpha[:].broadcast_to((P, N, P)), Alu.mult
                    )
                    ps = psp.tile([P, D], F32, tag="ps")
                    for n in range(N):
                        for h in range(D // MM_FREE):
                            nc.tensor.matmul(
                                ps[:, ts(h, MM_FREE)],
                                dgb[:, n, :],
                                vslice(halves, n)[:, ts(h, MM_FREE)],
                                start=(n == 0),
                                stop=(n == N - 1),
                            )
                    pending_store = (ps, t)

            # drain the last tile's deferred store
            pps, pt = pending_store
            osb = ob.tile([P, D], F32, tag="osb")
            nc.scalar.copy(osb[:], pps[:])
            nc.gpsimd.dma_start(out=o[ts(pt, P), :], in_=osb[:])

    orig = hw_specs.get_activation_tables
    bacc_orig = bacc.get_activation_tables
    try:
        hw_specs.get_activation_tables = _patched_tables(orig)
        bacc.get_activation_tables = hw_specs.get_activation_tables
        nc.finalize()
    finally:
        hw_specs.get_activation_tables = orig
        bacc.get_activation_tables = bacc_orig
    return nc


def _host_prep(V, rms_weight, w_proj):
    import ml_dtypes

    wc = (rms_weight.astype(np.float32) * w_proj.astype(np.float32)).astype(np.float32)
    wcb = np.ascontiguousarray(np.broadcast_to(wc, (P, D)))
    identb = np.ascontiguousarray(
        np.broadcast_to(
            np.eye(P, dtype=ml_dtypes.bfloat16), (N, P, P)
        ).transpose(1, 0, 2)
    )
    # [N, B*T, D] -> [B*T, N, D] so per-partition DMA runs are contiguous
    vt = np.ascontiguousarray(
        V.astype(np.float32).reshape(N, POS_TOTAL, D).transpose(1, 0, 2)
    )
    in_maps = []
    for c in range(N_CORES):
        shard = vt[c * POS_PER_CORE : (c + 1) * POS_PER_CORE]
        in_maps.append({"v": shard, "wcb": wcb, "identb": identb})
    return in_maps


def kernel(V, rms_weight, w_proj):
    if "nc" not in _CACHE:
        _CACHE["nc"] = _build()
    nc = _CACHE["nc"]
    in_maps = _host_prep(
        np.asarray(V), np.asarray(rms_weight), np.asarray(w_proj)
    )
    res = run_bass_kernel_spmd(nc, in_maps, core_ids=list(range(N_CORES)), trace=False)
    out = np.concatenate([res.results[c]["o"] for c in range(N_CORES)], axis=0)
    return out.reshape(B, T, D)


# revision 16
# speedup vs baseline: 1.0232x; 1.0232x over previous
"""Trainium2 Bass kernel for nn_AttnResBlock (RMSNorm -> scalar proj ->
softmax over depth N -> weighted sum of history).

Reference computation (per (b, t) position, D=1024, N=13):
  ms      = mean_d(V^2)
  logits  = rsqrt(ms + eps) * sum_d(V * (rms_weight * w_proj))
  alpha   = softmax_n(logits)
  out     = sum_n alpha_n * V_n

Sharding: B*T = 4096 positions split contiguously across 8 cores (512
positions each). All reductions are over D and N, both core-local -> no
collectives. Tiny [D] params are replicated (pre-broadcast on host).

Per-core layout: partitions = positions (tiles of 128), free dim = (n, d).
V is pre-transposed on the host to [pos, n, d]. Each tile's V is held as
two half-tiles (n 0:7 / 7:13) in separate pools so SBUF slots recycle at
half-tile granularity (the PE frees the first half mid-tile).

Measured per-slice engine costs (HW): DVE fused STT+accum ~1.36us, ACT
Square+accum ~1.42us, GpSimd bulk elementwise ~2.5us (DSP ucode - only
small ops live there). The 26 fused reduce slices are split:
  DVE : 13 w_proj dots + 1 square, softmax logits/max/recip/alpha
  ACT : 12 squares, rsqrt via ln/exp, softmax exp, PSUM evacuation
  GPS : diag-bank build (except last tile: DVE, to shorten the tail),
        SWDGE load groups + output stores
  PE  : weighted sum over N as 13 accumulating diag-matmuls in float32r
"""

import numpy as np

import concourse.bacc as bacc
import concourse.hw_specs as hw_specs
import concourse.mybir as mybir
from concourse.bass import ts
from concourse.bass_utils import run_bass_kernel_spmd
from concourse.tile import TileContext

N, B, T, D = 13, 2, 2048, 1024
N_CORES = 8
POS_TOTAL = B * T                    # 4096
POS_PER_CORE = POS_TOTAL // N_CORES  # 512
P = 128                              # SBUF partitions = positions per tile
TILES = POS_PER_CORE // P            # 4
EPS = float(np.finfo(np.float32).eps)
NSPLIT = 7                           # slices 0:7 in half A, 7:13 in half B

F32 = mybir.dt.float32
F32R = mybir.dt.float32r
BF16 = mybir.dt.bfloat16
Alu = mybir.AluOpType
Act = mybir.ActivationFunctionType

# load groups (slice ranges of n, ring): ring 0=sync HWDGE, 1=gpsimd SWDGE
LOAD_GROUPS = [(0, 4, 0), (4, 8, 1), (8, 11, 0), (11, 13, 0)]
# squares on DVE (rest of squares on ACT; all 13 dots on DVE)
DVE_SQS = ()
MM_FREE = 512  # free dim per matmul (2 per slice); PSUM bank limit for 4B

_CACHE = {}

_ACT_SET = "natural_log_exp_and_others"  # contains ln, exp, square, copy


def _patched_tables(orig):
    def fn(module_arch):
        t = orig(module_arch)
        return {k: (v if k == _ACT_SET else set()) for k, v in t.items()}

    return fn


def _build():
    nc = bacc.Bacc(None, target_bir_lowering=False)
    v = nc.dram_tensor("v", [POS_PER_CORE, N, D], F32R, kind="ExternalInput")
    wcb = nc.dram_tensor("wcb", [P, D], F32, kind="ExternalInput")
    identb = nc.dram_tensor("identb", [P, N, P], BF16, kind="ExternalInput")
    o = nc.dram_tensor("o", [POS_PER_CORE, D], F32, kind="ExternalOutput")

    with TileContext(nc) as tc:
        with (
            tc.tile_pool(name="cst", bufs=1) as cst,
            tc.tile_pool(name="vp", bufs=3) as vp,
            tc.tile_pool(name="sm", bufs=3) as sm,
            tc.tile_pool(name="dg", bufs=2) as dg,
            tc.tile_pool(name="ob", bufs=2) as ob,
            tc.tile_pool(name="ps", bufs=3, space="PSUM") as psp,
        ):
            wct = cst.tile([P, D], F32)
            idb = cst.tile([P, N, P], BF16)
            epst = cst.tile([P, 1], F32)

            rings = [nc.sync, nc.gpsimd]
            # wct gates every DVE dot -> absolutely first on the sync ring;
            # idb only gates the PE stage -> scalar ring (otherwise idle)
            nc.sync.dma_start(out=wct[:], in_=wcb[:, :])
            nc.scalar.dma_start(out=idb[:], in_=identb[:, :, :])
            nc.vector.memset(epst[:], EPS)
            pending_store = None  # deferred (psum, tile_idx) evac+store

            for t in range(TILES):
                # ---- load [128 pos, 13 n, 1024 d] in 4 grouped issues ----
                vt = vp.tile([P, N, D], F32R, tag="vt")
                for lo, hi, r in LOAD_GROUPS:
                    rings[r].dma_start(out=vt[:, lo:hi, :], in_=v[ts(t, P), lo:hi, :])

                # previous tile's PSUM evacuation + store, deferred here so
                # it never blocks this tile's stats behind the PE in the
                # in-order engine queues
                if pending_store is not None:
                    pps, pt = pending_store
                    osb = ob.tile([P, D], F32, tag="osb")
                    nc.scalar.copy(osb[:], pps[:])
                    nc.gpsimd.dma_start(out=o[ts(pt, P), :], in_=osb[:])
                    pending_store = None

                dotv = sm.tile([P, N], F32, tag="dotv")
                msv = sm.tile([P, N], F32, tag="msv")
                dumv = sm.tile([P, 1], F32, tag="dumv")
                trash = sm.tile([P, D], F32, tag="trash")
                for n in range(N):
                    vtn = vt[:, n, :].bitcast(F32)
                    # dot_n = sum_d V * (rms_weight * w_proj)   (fused DVE)
                    nc.vector.scalar_tensor_tensor(
                        out=dumv[:].broadcast_to((P, D)),
                        in0=vtn,
                        scalar=0.0,
                        in1=wct[:],
                        op0=Alu.bypass,
                        op1=Alu.mult,
                        accum_out=dotv[:, n : n + 1],
                    )
                    # ms_n = sum_d V^2
                    if n in DVE_SQS:
                        nc.vector.scalar_tensor_tensor(
                            out=dumv[:].broadcast_to((P, D)),
                            in0=vtn,
                            scalar=0.0,
                            in1=vtn,
                            op0=Alu.bypass,
                            op1=Alu.mult,
                            accum_out=msv[:, n : n + 1],
                        )
                    else:
                        nc.scalar.activation(
                            out=trash[:],
                            in_=vtn,
                            func=Act.Square,
                            accum_out=msv[:, n : n + 1],
                        )

                # ---- softmax over n (high priority: unblocks PE + buffers) ----
                with tc.high_priority(offset=100):
                    # rsqrt(mean + eps) = exp(-0.5 * ln(ms/D + eps))
                    lnv = sm.tile([P, N], F32, tag="lnv")
                    rsq = sm.tile([P, N], F32, tag="rsq")
                    nc.scalar.activation(
                        lnv[:], msv[:], Act.Ln, bias=epst[:], scale=1.0 / D
                    )
                    nc.scalar.activation(rsq[:], lnv[:], Act.Exp, scale=-0.5)
                    lg = sm.tile([P, N], F32, tag="lg")
                    nc.vector.tensor_tensor(lg[:], dotv[:], rsq[:], Alu.mult)
                    negm = sm.tile([P, 1], F32, tag="negm")
                    nc.vector.tensor_reduce(
                        out=negm[:], in_=lg[:], op=Alu.max,
                        axis=mybir.AxisListType.X, negate=True,
                    )
                    # exp with fused denominator accumulation
                    ev = sm.tile([P, N], F32, tag="ev")
                    ssum = sm.tile([P, 1], F32, tag="ssum")
                    nc.scalar.activation(
                        ev[:], lg[:], Act.Exp, bias=negm[:], accum_out=ssum[:]
                    )
                    rcp = sm.tile([P, 1], F32, tag="rcp")
                    nc.vector.reciprocal(rcp[:], ssum[:])
                    alpha = sm.tile([P, N, 1], F32, tag="alpha")
                    nc.vector.tensor_scalar(
                        alpha[:, :, 0], ev[:], rcp[:], None, Alu.mult
                    )

                    # ---- weighted sum over n: PE with diagonal-alpha stationary ----
                    dgb = dg.tile([P, N, P], F32R, tag="dgb")
                    dgb_eng = nc.vector if t == TILES - 1 else nc.gpsimd
                    dgb_eng.tensor_tensor(
                        dgb[:], idb[:], alpha[:].broadcast_to((P, N, P)), Alu.mult
                    )
                    ps = psp.tile([P, D], F32, tag="ps")
                    for n in range(N):
                        for h in range(D // MM_FREE):
                            nc.tensor.matmul(
                                ps[:, ts(h, MM_FREE)],
                                dgb[:, n, :],
                                vt[:, n, ts(h, MM_FREE)],
                                start=(n == 0),
                                stop=(n == N - 1),
                            )
                    pending_store = (ps, t)

            # drain the last tile's deferred store
            pps, pt = pending_store
            osb = ob.tile([P, D], F32, tag="osb")
            nc.scalar.copy(osb[:], pps[:])
            nc.gpsimd.dma_start(out=o[ts(pt, P), :], in_=osb[:])

    orig = hw_specs.get_activation_tables
    bacc_orig = bacc.get_activation_tables
    try:
        hw_specs.get_activation_tables = _patched_tables(orig)
        bacc.get_activation_tables = hw_specs.get_activation_tables
        nc.finalize()
    finally:
        hw_specs.get_activation_tables = orig
        bacc.get_activation_tables = bacc_orig
    return nc


def _host_prep(V, rms_weight, w_proj):
    import ml_dtypes

    wc = (rms_weight.astype(np.float32) * w_proj.astype(np.float32)).astype(np.float32)
    wcb = np.ascontiguousarray(np.broadcast_to(wc, (P, D)))
    identb = np.ascontiguousarray(
        np.broadcast_to(
            np.eye(P, dtype=ml_dtypes.bfloat16), (N, P, P)
        ).transpose(1, 0, 2)
    )
    # [N, B*T, D] -> [B*T, N, D] so per-partition DMA runs are contiguous
    vt = np.ascontiguousarray(
        V.astype(np.float32).reshape(N, POS_TOTAL, D).transpose(1, 0, 2)
    )
    in_maps = []
    for c in range(N_CORES):
        shard = vt[c * POS_PER_CORE : (c + 1) * POS_PER_CORE]
        in_maps.append({"v": shard, "wcb": wcb, "identb": identb})
    return in_maps


def kernel(V, rms_weight, w_proj):
    if "nc" not in _CACHE:
        _CACHE["nc"] = _build()
    nc = _CACHE["nc"]
    in_maps = _host_prep(
        np.asarray(V), np.asarray(rms_weight), np.asarray(w_proj)
    )
    res = run_bass_kernel_spmd(nc, in_maps, core_ids=list(range(N_CORES)), trace=False)
    out = np.concatenate([res.results[c]["o"] for c in range(N_CORES)], axis=0)
    return out.reshape(B, T, D)
